# revision 18
# baseline (speedup 1.0000x reference)
import sys

if "/opt/trn_rl_repo" not in sys.path:
    sys.path.insert(0, "/opt/trn_rl_repo")

import numpy as np
import ml_dtypes

import concourse.bass as bass
import concourse.bacc as bacc
import concourse.mybir as mybir
import concourse.tile as tile
from concourse.bass_utils import run_bass_kernel_spmd

BF16 = mybir.dt.bfloat16
F32 = mybir.dt.float32
NPBF16 = ml_dtypes.bfloat16

# problem dims (hardcoded; see nn_GTO_Atten_76974403879664)
B = 4            # batch
N = 8192         # nodes per batch element
C = 512          # channels
H = 8            # heads
TD = 64          # head dim
M = 256          # n_token
SCALE = 1.0 / 8.0  # 1/sqrt(TD)

# per-core sharding: core c -> (bi = c//2, hg = c%2)
# stage 1: 4 heads (hg*4 .. hg*4+3) over all N nodes
# stage 2: all 8 heads over NL = N//2 nodes (host pre-swaps halves for odd
# cores so each core's stage-2 nodes are columns 0..NL-1 of its w0t input)
NL = N // 2      # 4096
HL = H // 2      # 4 heads per core in stage 1


def _build_nc():
    nc = bacc.Bacc(None)

    w0t = nc.declare_dram_parameter("w0t", [C, N], BF16, isOutput=False)
    kv1w_p = nc.declare_dram_parameter("kv1w_p", [C + 1, 256], BF16, isOutput=False)
    kv1w4 = nc.declare_dram_parameter("kv1w4", [C + 1, 260], BF16, isOutput=False)
    qst = nc.declare_dram_parameter("qst", [2 * TD, HL * M], BF16, isOutput=False)
    q3w = nc.declare_dram_parameter("q3w", [C + 1, C], BF16, isOutput=False)
    qkv2k = nc.declare_dram_parameter("qkv2k", [TD, TD], BF16, isOutput=False)
    qkv2v = nc.declare_dram_parameter("qkv2v", [TD + 1, TD + 1], BF16, isOutput=False)
    projw = nc.declare_dram_parameter("projw", [C + 1, C], BF16, isOutput=False)
    biasesT = nc.declare_dram_parameter("biasesT", [128, 10], F32, isOutput=False)
    out = nc.declare_dram_parameter("out", [C, NL], F32, isOutput=True)

    with tile.TileContext(nc) as tc:
        with (
            tc.tile_pool(name="wta", bufs=1) as wta_pool,
            tc.tile_pool(name="sh8k", bufs=4) as sh8k_pool,   # wtB then wT reuse
            tc.tile_pool(name="kv1t_q3t", bufs=4) as kq_pool,  # kv1t then q3t reuse
            tc.tile_pool(name="kv1n", bufs=1) as kv1n_pool,
            tc.tile_pool(name="wgt", bufs=1) as wgt_pool,
            tc.tile_pool(name="expl", bufs=4) as expl_pool,
            tc.tile_pool(name="el2", bufs=4) as el2_pool,
            tc.tile_pool(name="sm", bufs=2) as sm_pool,
            tc.tile_pool(name="stage", bufs=3) as stage_pool,
            tc.tile_pool(name="pbig", bufs=2, space="PSUM") as pbig_pool,
            tc.tile_pool(name="pacc", bufs=2, space="PSUM") as pacc_pool,
            tc.tile_pool(name="pev", bufs=2, space="PSUM") as pev_pool,
            tc.tile_pool(name="dram", bufs=1, space="DRAM") as dram_pool,
        ):
            # ---- load W0T halves (bf16): wta = cols 0:NL (persistent),
            #      wtb = cols NL:N (stage-1 only, slots later reused by wT) ----
            wta = [wta_pool.tile([128, NL], BF16, tag=f"wta{k}", name=f"wta{k}")
                   for k in range(4)]
            wtb = [sh8k_pool.tile([128, NL], BF16, tag="sh8k", name=f"wtb{k}")
                   for k in range(4)]

            def wt_cols(k, j512):
                # [128, 512] slice of W0T k-tile at column block j512 (0..15)
                if j512 < 8:
                    return wta[k][:, j512 * 512:(j512 + 1) * 512]
                return wtb[k][:, (j512 - 8) * 512:(j512 - 7) * 512]

            def wt_chunk(k, i):
                # [128, 128] slice of W0T k-tile at chunk i (0..63)
                if i < 32:
                    return wta[k][:, i * 128:(i + 1) * 128]
                return wtb[k][:, (i - 32) * 128:(i - 31) * 128]

            # ---- ones row (for bias matmuls) ----
            ones_t = wgt_pool.tile([1, 512], BF16, tag="ones", name="ones")
            nc.vector.memset(ones_t[:], 1.0)

            # ---- load weights ----
            kv1w_t = [wgt_pool.tile([128, 256], BF16, tag=f"kv1w{k}", name=f"kv1w{k}")
                      for k in range(4)]
            for k in range(4):
                nc.sync.dma_start(kv1w_t[k][:], kv1w_p[k * 128:(k + 1) * 128, :])
            biases_t = wgt_pool.tile([128, 10], F32, tag="biasesT", name="biases_t")
            nc.sync.dma_start(biases_t[:], biasesT[:, :])

            kv1w4_t = [wgt_pool.tile([128, 260], BF16, tag=f"kv1w4{k}", name=f"kv1w4{k}")
                       for k in range(4)]
            for k in range(4):
                nc.sync.dma_start(kv1w4_t[k][:], kv1w4[k * 128:(k + 1) * 128, :])
            kv1w4_b = wgt_pool.tile([1, 260], BF16, tag="kv1w4b", name="kv1w4b")
            nc.sync.dma_start(kv1w4_b[:], kv1w4[C:C + 1, :])

            qst_t = wgt_pool.tile([2 * TD, HL * M], BF16, tag="qst", name="qst")
            nc.sync.dma_start(qst_t[:], qst[:, :])

            # w0t first half right after the stage-1 weights, column-major
            # 1024-col blocks so the first matmuls start early
            for cb in range(4):
                for k in range(4):
                    nc.sync.dma_start(
                        wta[k][:, cb * 1024:(cb + 1) * 1024],
                        w0t[k * 128:(k + 1) * 128, cb * 1024:(cb + 1) * 1024])

            q3w_t = [wgt_pool.tile([128, C], BF16, tag=f"q3w{k}", name=f"q3w{k}")
                     for k in range(4)]
            for k in range(4):
                nc.sync.dma_start(q3w_t[k][:], q3w[k * 128:(k + 1) * 128, :])

            qkv2k_t = wgt_pool.tile([TD, TD], BF16, tag="qkv2k", name="qkv2k")
            nc.sync.dma_start(qkv2k_t[:], qkv2k[:, :])
            qkv2v_t = wgt_pool.tile([TD + 1, TD + 1], BF16, tag="qkv2v", name="qkv2v")
            nc.sync.dma_start(qkv2v_t[:], qkv2v[:, :])

            projw_t = [wgt_pool.tile([128, C], BF16, tag=f"projw{k}", name=f"projw{k}")
                       for k in range(4)]
            for k in range(4):
                nc.sync.dma_start(projw_t[k][:], projw[k * 128:(k + 1) * 128, :])
            for cb in range(4):
                for k in range(4):
                    nc.sync.dma_start(
                        wtb[k][:, cb * 1024:(cb + 1) * 1024],
                        w0t[k * 128:(k + 1) * 128, NL + cb * 1024:NL + (cb + 1) * 1024])

            # ---- stage 1 precompute: kv1T (2 head-pairs) and kv1n ----
            # kv1t[p]: [128, N] rows 0-63 head 2p, 64-127 head 2p+1 (local)
            # kv1t[p][half]: [128, NL] (4 tiles share slots with the later q3t)
            kv1t = [[kq_pool.tile([128, NL], BF16, tag="kq", name=f"kv1t{p}_{hf}")
                     for hf in range(2)] for p in range(2)]
            kv1n = kv1n_pool.tile([128, 64 * 260], BF16, tag="kv1n", name="kv1n")
            # interleaved by 512-column region: kv1t (both pairs) then the 4
            # kv1n chunks of that region, so stage-1 attention pipelines in
            # right behind the precompute front
            for j in range(N // 512):
                for p in range(2):
                    ps = pev_pool.tile([128, 512], F32, tag="pev", name="pev")
                    for k in range(4):
                        nc.tensor.matmul(
                            ps[:],
                            kv1w_t[k][:, p * 128:(p + 1) * 128],
                            wt_cols(k, j),
                            start=(k == 0), stop=(k == 3),
                        )
                    nc.scalar.add(
                        kv1t[p][j // 8][:, (j % 8) * 512:(j % 8 + 1) * 512], ps[:],
                        biases_t[:, p:p + 1])
                for i in range(4 * j, 4 * j + 4):
                    ps = pev_pool.tile([128, 260], F32, tag="pev", name="pev")
                    for k in range(4):
                        nc.tensor.matmul(
                            ps[:],
                            wt_chunk(k, i),
                            kv1w4_t[k][:],
                            start=(k == 0), stop=False,
                        )
                    nc.tensor.matmul(
                        ps[:],
                        ones_t[:, 0:128],
                        kv1w4_b[:],
                        start=False, stop=True,
                    )
                    nc.vector.tensor_copy(kv1n[:, i * 260:(i + 1) * 260], ps[:])

            # q3T emitter: interleaved into stage-1 attention so the q3t
            # matmuls fill PE idle while stage-1 softmax runs on ACT
            q3ts = [None] * 4

            def make_q3t(p2):
                q3t = kq_pool.tile([128, NL], BF16, tag="kq", name=f"q3t{p2}")
                for j in range(NL // 512):
                    ps = pev_pool.tile([128, 512], F32, tag="pev", name="pev")
                    for k in range(4):
                        nc.tensor.matmul(
                            ps[:],
                            q3w_t[k][:, p2 * 128:(p2 + 1) * 128],
                            wta[k][:, j * 512:(j + 1) * 512],
                            start=(k == 0), stop=(k == 3),
                        )
                    nc.vector.tensor_scalar_add(q3t[:, j * 512:(j + 1) * 512], ps[:],
                                                biases_t[:, 2 + p2:3 + p2])
                q3ts[p2] = q3t

            # ---- stage 1 attention (per local head): logits^T -> exp -> ptT ----
            cc_in = [dram_pool.tile([2 * TD, M], BF16, name=f"cc_in{a}")
                     for a in range(2)]
            cc_out = [dram_pool.tile([4 * TD, M], BF16, name=f"cc_out{a}")
                      for a in range(2)]
            for h in range(HL):
                pt_ps = pacc_pool.tile([TD + 1, M], F32, tag="pacc", name="pacc")
                for g in range(16):  # groups of 4 chunks
                    lg = pbig_pool.tile([128, 4 * M], F32, tag="pbig", name="pbig")
                    for q in range(4):
                        i = 4 * g + q
                        nc.tensor.matmul(
                            lg[:, q * M:(q + 1) * M],
                            kv1t[h // 2][i // 32][(h % 2) * 64:(h % 2) * 64 + 64,
                                                  (i % 32) * 128:(i % 32 + 1) * 128],
                            qst_t[(h % 2) * 64:(h % 2) * 64 + 64,
                                  h * M:(h + 1) * M],
                            start=True, stop=True,
                        )
                    ex = expl_pool.tile([128, 4 * M], BF16, tag="expl", name="expl")
                    nc.scalar.activation(ex[:], lg[:], mybir.ActivationFunctionType.Exp)
                    for q in range(4):
                        i = 4 * g + q
                        nc.tensor.matmul(
                            pt_ps[:],
                            kv1n[:, i * 260 + h * 65: i * 260 + h * 65 + 65],
                            ex[:, q * M:(q + 1) * M],
                            start=(i == 0), stop=(i == 63),
                        )
                # normalize: ptn = pt[0:64] / pt[64]
                rs = sm_pool.tile([1, M], F32, tag="rs", name="rs")
                nc.vector.tensor_copy(rs[:], pt_ps[TD:TD + 1, :])
                rr = sm_pool.tile([1, M], F32, tag="rr", name="rr")
                nc.vector.reciprocal_approx_fast(rr[:], rs[:])
                rb = sm_pool.tile([TD, M], F32, tag="rb", name="rb")
                nc.gpsimd.partition_broadcast(rb[:], rr[:])
                ptn = sm_pool.tile([TD, M], BF16, tag="ptn", name="ptn")
                nc.vector.tensor_mul(ptn[:], pt_ps[0:TD, :], rb[:])
                nc.sync.dma_start(cc_in[h // 2][(h % 2) * TD:(h % 2) * TD + TD, :],
                                  ptn[:])
                if h % 2 == 1:
                    nc.gpsimd.collective_compute(
                        "AllGather",
                        mybir.AluOpType.bypass,
                        replica_groups=[[0, 1], [2, 3], [4, 5], [6, 7]],
                        ins=[cc_in[h // 2].opt()],
                        outs=[cc_out[h // 2].opt()],
                    )
                    make_q3t(h - 1)
                    make_q3t(h)

            # ptg[h8]: [65, M] rows 0-63 ptT head h8, row 64 ones.
            # AG a gathers local heads {2a, 2a+1}:
            #   out rows 0:128   -> even core's heads = global heads 2a, 2a+1
            #   out rows 128:256 -> odd core's heads  = global heads 4+2a, 4+2a+1
            ptg = [None] * H
            for a in range(2):
                for r in range(4):
                    h8 = (4 if r >= 2 else 0) + 2 * a + (r % 2)
                    t = sm_pool.tile([TD + 1, M], BF16, tag=f"ptg{h8}",
                                     name=f"ptg{h8}", bufs=1)
                    nc.sync.dma_start(t[0:TD, :],
                                      cc_out[a][r * TD:(r + 1) * TD, :])
                    nc.vector.memset(t[TD:TD + 1, :], 1.0)
                    ptg[h8] = t

            # ---- stage 2 ----
            # wT[p]: [128, NL] attention output (pre-projection), channel-major;
            # slots reused from wtb
            wT = [sh8k_pool.tile([128, NL], BF16, tag="sh8k", name=f"wT{p}")
                  for p in range(4)]

            kts = [None] * H
            vvs = [None] * H

            def make_ktv(h8):
                # kT: [128, M] (duplicated in both partition halves)
                ps_k = pev_pool.tile([TD, M], F32, tag="pev", name="pev")
                nc.tensor.matmul(ps_k[:], qkv2k_t[:], ptg[h8][0:TD, :],
                                 start=True, stop=True)
                kt = sm_pool.tile([2 * TD, M], BF16, tag=f"kt{h8}", name=f"kt{h8}",
                                  bufs=1)
                nc.vector.tensor_copy(kt[0:TD, :], ps_k[:])
                nc.vector.tensor_copy(kt[TD:2 * TD, :], ps_k[:])
                kts[h8] = kt
                vv = []
                for mt in range(2):
                    ps_v = pev_pool.tile([128, TD + 1], F32, tag="pev", name="pev")
                    nc.tensor.matmul(ps_v[:], ptg[h8][:, mt * 128:(mt + 1) * 128],
                                     qkv2v_t[:], start=True, stop=True)
                    v_t = sm_pool.tile([128, TD + 1], BF16, tag=f"vv{h8}_{mt}",
                                       name=f"vv{h8}_{mt}", bufs=1)
                    nc.vector.tensor_copy(v_t[:], ps_v[:])
                    vv.append(v_t)
                vvs[h8] = vv

            def attn_block(h8, j, alt):
                # logits^T for both m-tiles of head h8 at n-block j -> exp ->
                # value matmul -> normalized wT rows
                p2, hh = h8 // 2, h8 % 2
                lg = pbig_pool.tile([128, 1024], F32, tag="pbig", name="pbig")
                for mt in range(2):
                    nc.tensor.matmul(
                        lg[:, mt * 512:(mt + 1) * 512],
                        kts[h8][hh * 64:hh * 64 + 64, mt * 128:(mt + 1) * 128],
                        q3ts[p2][hh * 64:hh * 64 + 64, j * 512:(j + 1) * 512],
                        start=True, stop=True,
                    )
                el2 = el2_pool.tile([128, 1024], BF16, tag="el2", name="el2")
                nc.scalar.activation(el2[:], lg[:], mybir.ActivationFunctionType.Exp)
                ps_w = pacc_pool.tile([TD + 1, 512], F32, tag="pacc", name="pacc")
                for mt in range(2):
                    nc.tensor.matmul(
                        ps_w[:],
                        vvs[h8][mt][:],
                        el2[:, mt * 512:(mt + 1) * 512],
                        start=(mt == 0), stop=(mt == 1),
                    )
                rs2 = sm_pool.tile([1, 512], F32, tag="rs2", name="rs2")
                if alt:
                    nc.scalar.copy(rs2[:], ps_w[TD:TD + 1, :])
                else:
                    nc.vector.tensor_copy(rs2[:], ps_w[TD:TD + 1, :])
                rr2 = sm_pool.tile([1, 512], F32, tag="rr2", name="rr2")
                nc.vector.reciprocal_approx_fast(rr2[:], rs2[:])
                rb2 = sm_pool.tile([TD, 512], F32, tag="rb2", name="rb2")
                nc.gpsimd.partition_broadcast(rb2[:], rr2[:])
                nc.vector.tensor_mul(
                    wT[p2][hh * 64:hh * 64 + 64, j * 512:(j + 1) * 512],
                    ps_w[0:TD, :], rb2[:])

            # pass A: AG1-covered heads (0,1,4,5), n-block outer
            for h8 in (0, 1, 4, 5):
                make_ktv(h8)
            for j in range(NL // 512):
                for idx, h8 in enumerate((0, 1, 4, 5)):
                    attn_block(h8, j, alt=(idx % 2 == 0))

            # pass B: AG2-covered heads + final projection per n-block
            for h8 in (2, 3, 6, 7):
                make_ktv(h8)
            for j in range(NL // 512):
                for idx, h8 in enumerate((2, 3, 6, 7)):
                    attn_block(h8, j, alt=(idx % 2 == 0))
                for ot in range(4):
                    ps = pev_pool.tile([128, 512], F32, tag="pev", name="pev")
                    for k in range(4):
                        nc.tensor.matmul(
                            ps[:],
                            projw_t[k][:, ot * 128:(ot + 1) * 128],
                            wT[k][:, j * 512:(j + 1) * 512],
                            start=(k == 0), stop=(k == 3),
                        )
                    st = stage_pool.tile([128, 512], F32, tag="stage", name="stage")
                    if ot % 2 == 0:
                        nc.vector.tensor_scalar_add(st[:], ps[:],
                                                    biases_t[:, 6 + ot:7 + ot])
                    else:
                        nc.scalar.add(st[:], ps[:], biases_t[:, 6 + ot:7 + ot])
                    nc.sync.dma_start(
                        out[ot * 128:(ot + 1) * 128, j * 512:(j + 1) * 512], st[:])

    nc.finalize()
    return nc


_NC_CACHE = None


def _get_nc():
    global _NC_CACHE
    if _NC_CACHE is None:
        _NC_CACHE = _build_nc()
    return _NC_CACHE


def _prep_inputs(W0, Q, kv1_w, kv1_b, qkv2_w, q3_w, q3_b, proj_w, proj_b):
    W0 = np.asarray(W0, dtype=np.float32)
    Q = np.asarray(Q, dtype=np.float32)
    kv1_w = np.asarray(kv1_w, dtype=np.float32)
    kv1_b = np.asarray(kv1_b, dtype=np.float32)
    qkv2_w = np.asarray(qkv2_w, dtype=np.float32)
    q3_w = np.asarray(q3_w, dtype=np.float32)
    q3_b = np.asarray(q3_b, dtype=np.float32)
    proj_w = np.asarray(proj_w, dtype=np.float32)
    proj_b = np.asarray(proj_b, dtype=np.float32)

    q3w_np = np.concatenate([q3_w, q3_b[None, :]], axis=0).astype(NPBF16)
    projw_np = np.concatenate([proj_w, proj_b[None, :]], axis=0).astype(NPBF16)
    qkv2k_np = (qkv2_w[:, :TD] * SCALE).astype(NPBF16)
    qkv2v_np = np.zeros((TD + 1, TD + 1), np.float32)
    qkv2v_np[:TD, :TD] = qkv2_w[:, TD:]
    qkv2v_np[TD, TD] = 1.0
    qkv2v_np = qkv2v_np.astype(NPBF16)

    in_maps = []
    for c in range(8):
        bi, hg = c // 2, c % 2
        w0c = W0[bi]
        if hg == 1:
            w0c = np.concatenate([w0c[NL:], w0c[:NL]], axis=0)
        w0t_np = np.ascontiguousarray(w0c.T).astype(NPBF16)

        kv1w_p_np = np.empty((C + 1, 256), np.float32)
        kv1w_p_np[:C] = kv1_w[:, hg * 256:(hg + 1) * 256]
        kv1w_p_np[C] = kv1_b[hg * 256:(hg + 1) * 256]

        kv1w4_np = np.zeros((C + 1, 260), np.float32)
        for h in range(HL):
            gh = hg * HL + h
            kv1w4_np[:C, h * 65:h * 65 + TD] = kv1_w[:, gh * TD:(gh + 1) * TD]
            kv1w4_np[C, h * 65:h * 65 + TD] = kv1_b[gh * TD:(gh + 1) * TD]
            kv1w4_np[C, h * 65 + TD] = 1.0

        qst_np = np.empty((2 * TD, HL * M), np.float32)
        for h in range(HL):
            qst_np[0:TD, h * M:(h + 1) * M] = (Q[hg * HL + h] * SCALE).T
        qst_np[TD:2 * TD] = qst_np[0:TD]

        biasesT_np = np.zeros((128, 10), np.float32)
        for p in range(2):
            biasesT_np[:, p] = kv1_b[hg * 256 + p * 128: hg * 256 + (p + 1) * 128]
        for p2 in range(4):
            biasesT_np[:, 2 + p2] = q3_b[p2 * 128:(p2 + 1) * 128]
        for ot in range(4):
            biasesT_np[:, 6 + ot] = proj_b[ot * 128:(ot + 1) * 128]

        in_maps.append({
            "w0t": w0t_np,
            "biasesT": biasesT_np,
            "kv1w_p": kv1w_p_np.astype(NPBF16),
            "kv1w4": kv1w4_np.astype(NPBF16),
            "qst": qst_np.astype(NPBF16),
            "q3w": q3w_np,
            "qkv2k": qkv2k_np,
            "qkv2v": qkv2v_np,
            "projw": projw_np,
        })
    return in_maps


def run(trace=False, **inputs):
    nc = _get_nc()
    in_maps = _prep_inputs(**inputs)
    res = run_bass_kernel_spmd(nc, in_maps, core_ids=list(range(8)), trace=trace)
    out_full = np.empty((B, N, C), np.float32)
    for c in range(8):
        bi, hg = c // 2, c % 2
        out_full[bi, hg * NL:(hg + 1) * NL, :] = res.results[c]["out"].T
    return out_full, res


def kernel(**inputs):
    out, _ = run(trace=False, **inputs)
    return out
